# revision 9
# baseline (speedup 1.0000x reference)
"""Trainium2 Bass kernel for nn_Attn_47768626266275.

Computation (reference):
    energy[b,s,:] = W @ enc[b,s,:] + bias          # nn.Linear
    scores[b,s]   = hidden[b,:] . energy[b,s,:]
    out           = softmax(scores, axis=-1)[:, None, :]

Algebraic rewrite used here:
    scores[b,s] = enc[b,s,:] . v[b,:] + c[b],  v = hidden @ W,  c = hidden . bias
    softmax is shift-invariant along s, so c[b] drops out entirely.

This turns the [B*S,H]x[H,H] projection (137 GFLOP) into a [B,H]x[H,H] matmul
plus a streamed per-row dot product -> the kernel is HBM-bound on reading
encoder_outputs exactly once (33.5 MB/core across 8 cores).

Sharding: data-parallel over batch. Core i handles batches [4i, 4i+4).
No collectives. W is replicated (4 MB/core). hidden is passed pre-transposed
from the host (16 KB) so it can be used directly as the matmul stationary
operand.

Per-core pipeline:
  - DMA W -> SBUF, hiddenT -> SBUF
  - PE: v = hiddenT.T @ W                       [4, 1024] PSUM
  - PE: broadcast v[b] to all 128 partitions    (ones outer product)
  - stream enc in [128, 8, 1024] supertiles (4 MB DMAs, 4 KB descriptors);
    DVE tensor_tensor_reduce does mult+reduce in one pass:
        scores[p, c] = sum_h enc_tile[p, c, h] * v_b[p, h]
  - softmax over the [128, 16] score tile per batch:
        row-max (DVE) -> PE transpose -> global max -> ACT exp w/ accum ->
        PE ones-matmul partition sum -> DVE reciprocal -> scale -> PE
        transpose -> DMA out (contiguous 512B rows)
"""

import numpy as np

import concourse.bass as bass
import concourse.bacc as bacc
import concourse.tile as tile
from concourse import mybir
from concourse.masks import make_identity

B = 32          # full batch
S = 2048        # sequence
H = 1024        # hidden
NCORES = 8
BPC = B // NCORES   # batches per core = 4
NU = 2          # supertiles per batch
NT = 8          # 128-row subtiles per supertile
NC_P = 128      # partitions
KCH = H // NC_P  # 8 contraction chunks for the v matmul

F32 = mybir.dt.float32

_CACHED = {}


def _build_bass():
    from contextlib import ExitStack

    nc = bacc.Bacc()

    enc_h = nc.declare_dram_parameter("enc", [BPC, S, H], F32, isOutput=False)
    hT_h = nc.declare_dram_parameter("hT", [H, BPC], F32, isOutput=False)
    w_h = nc.declare_dram_parameter("W", [H, H], F32, isOutput=False)
    out_h = nc.declare_dram_parameter("out", [BPC, S], F32, isOutput=True)

    with tile.TileContext(nc) as tc, ExitStack() as ctx:
        _emit(ctx, tc, enc_h, hT_h, w_h, out_h)
    return nc


def _emit(ctx, tc, enc_h, hT_h, w_h, out_h):
    nc = tc.nc

    singles = ctx.enter_context(tc.tile_pool(name="singles", bufs=1))
    encp = ctx.enter_context(tc.tile_pool(name="encp", bufs=3))
    scratchp = ctx.enter_context(tc.tile_pool(name="scratchp", bufs=2))
    scoresp = ctx.enter_context(tc.tile_pool(name="scoresp", bufs=2))
    smallp = ctx.enter_context(tc.tile_pool(name="smallp", bufs=4))
    pmm = ctx.enter_context(tc.tile_pool(name="pmm", bufs=2, space="PSUM"))
    psmall = ctx.enter_context(tc.tile_pool(name="psmall", bufs=1, space="PSUM"))

    # ---- constants -------------------------------------------------------
    ident = singles.tile([NC_P, NC_P], F32, tag="ident")
    make_identity(nc, ident)
    ones_col = singles.tile([1, NC_P], F32, tag="ones_col")   # lhsT for bcast
    nc.vector.memset(ones_col, 1.0)
    ones_sum = singles.tile([NC_P, 1], F32, tag="ones_sum")   # rhs for P-sum
    nc.vector.memset(ones_sum, 1.0)
    # sel[:, b, :] is a [BPC, 128] stationary matrix whose row b is all-ones:
    # matmul(lhsT=sel[:,b,:], rhs=v_sb) broadcasts v[b,:] to all partitions.
    sel = singles.tile([BPC, BPC, NC_P], F32, tag="sel")
    nc.gpsimd.memset(sel, 0.0)
    nc.gpsimd.affine_select(
        out=sel,
        in_=sel,
        compare_op=mybir.AluOpType.not_equal,
        fill=1.0,
        base=0,
        # expr = p - b  -> fill 1.0 where p == b
        pattern=[[-1, BPC], [0, NC_P]],
        channel_multiplier=1,
    )

    # ---- load W and hiddenT ---------------------------------------------
    w_sb = singles.tile([NC_P, KCH, H], F32, tag="w_sb")
    nc.sync.dma_start(out=w_sb, in_=w_h[:].rearrange("(k p) h -> p k h", p=NC_P))

    hT_sb = singles.tile([NC_P, KCH, BPC], F32, tag="hT_sb")
    nc.sync.dma_start(out=hT_sb, in_=hT_h[:].rearrange("(k p) b -> p k b", p=NC_P))

    # ---- v = hiddenT.T @ W  -> [BPC, H] ---------------------------------
    v_ps = pmm.tile([BPC, H], F32, tag="mm")
    for half in range(2):
        cols = slice(half * 512, (half + 1) * 512)
        for k in range(KCH):
            nc.tensor.matmul(
                v_ps[:, cols],
                lhsT=hT_sb[:, k, :],
                rhs=w_sb[:, k, cols],
                start=(k == 0),
                stop=(k == KCH - 1),
            )
    v_sb = singles.tile([BPC, H], F32, tag="v_sb")
    nc.scalar.copy(v_sb, v_ps)

    # ---- broadcast v[b] across all 128 partitions -----------------------
    vb_sb = []
    for b in range(BPC):
        vb_ps = pmm.tile([NC_P, H], F32, tag="mm")
        for half in range(2):
            cols = slice(half * 512, (half + 1) * 512)
            nc.tensor.matmul(
                vb_ps[:, cols],
                lhsT=sel[:, b, :],
                rhs=v_sb[:, cols],
                start=True,
                stop=True,
            )
        t = singles.tile([NC_P, H], F32, tag=f"vb{b}")
        nc.scalar.copy(t, vb_ps)
        vb_sb.append(t)

    # ---- main stream: scores + softmax ----------------------------------
    enc_ap = enc_h[:].rearrange("b (u t p) h -> b u p t h", u=NU, t=NT, p=NC_P)
    out_ap = out_h[:].rearrange("b (c p) -> b c p", p=NC_P)  # c = u*NT + t

    for b in range(BPC):
        scores = scoresp.tile([NC_P, NU * NT], F32, tag="scores")
        for u in range(NU):
            e_sb = encp.tile([NC_P, NT, H], F32, tag="enc")
            nc.sync.dma_start(out=e_sb, in_=enc_ap[b, u])
            scratch = scratchp.tile([NC_P, H], F32, tag="scratch")
            for t in range(NT):
                c = u * NT + t
                # fused dot product: out = (enc * 1.0) * v; accum = sum(out)
                nc.vector.scalar_tensor_tensor(
                    out=scratch,
                    in0=e_sb[:, t, :],
                    scalar=1.0,
                    in1=vb_sb[b],
                    op0=mybir.AluOpType.mult,
                    op1=mybir.AluOpType.mult,
                    accum_out=scores[:, c : c + 1],
                )

        # ---- softmax over all 2048 scores of batch b --------------------
        ncols = NU * NT
        rmax = smallp.tile([NC_P, 1], F32, tag="rmax")
        nc.vector.tensor_reduce(
            out=rmax, in_=scores, axis=mybir.AxisListType.X, op=mybir.AluOpType.max
        )
        rmaxT_ps = psmall.tile([1, NC_P], F32, tag="ps_a")
        nc.tensor.transpose(rmaxT_ps, rmax, ident)
        rmaxT = smallp.tile([1, NC_P], F32, tag="rmaxT")
        nc.scalar.copy(rmaxT, rmaxT_ps)
        gmax = smallp.tile([1, 1], F32, tag="gmax")
        nc.vector.tensor_reduce(
            out=gmax, in_=rmaxT, axis=mybir.AxisListType.X, op=mybir.AluOpType.max
        )
        # broadcast -max to all partitions
        gmax_ps = psmall.tile([NC_P, 1], F32, tag="ps_b")
        nc.tensor.matmul(gmax_ps, lhsT=ones_col, rhs=gmax, start=True, stop=True)
        negmax = smallp.tile([NC_P, 1], F32, tag="negmax")
        nc.scalar.mul(negmax, gmax_ps, -1.0)

        probs = scoresp.tile([NC_P, ncols], F32, tag="probs")
        ssum = smallp.tile([NC_P, 1], F32, tag="ssum")
        nc.scalar.activation(
            out=probs,
            in_=scores,
            func=mybir.ActivationFunctionType.Exp,
            bias=negmax,
            scale=1.0,
            accum_out=ssum,
        )
        tot_ps = psmall.tile([1, 1], F32, tag="ps_c")
        nc.tensor.matmul(tot_ps, lhsT=ssum, rhs=ones_sum, start=True, stop=True)
        rinv = smallp.tile([1, 1], F32, tag="rinv")
        nc.vector.reciprocal(rinv, tot_ps)
        rinv_ps = psmall.tile([NC_P, 1], F32, tag="ps_b")
        nc.tensor.matmul(rinv_ps, lhsT=ones_col, rhs=rinv, start=True, stop=True)
        rinv_b = smallp.tile([NC_P, 1], F32, tag="rinv_b")
        nc.scalar.copy(rinv_b, rinv_ps)
        nc.vector.tensor_scalar_mul(out=probs, in0=probs, scalar1=rinv_b)

        pT_ps = psmall.tile([ncols, NC_P], F32, tag="ps_d")
        nc.tensor.transpose(pT_ps, probs, ident)
        pT = scoresp.tile([ncols, NC_P], F32, tag="pT")
        nc.scalar.copy(pT, pT_ps)
        nc.sync.dma_start(out=out_ap[b], in_=pT)


def _get_nc():
    if "nc" not in _CACHED:
        nc = _build_bass()
        # Bacc defers register allocation etc. to finalize(); the PJRT run
        # path serializes the module as-is, so legalize it here.
        nc.finalize()
        _CACHED["nc"] = nc
    return _CACHED["nc"]


def run(hidden, encoder_outputs, W, trace=False):
    """Shard, run on 8 cores, gather. Returns (out [B,1,S], BassKernelResults)."""
    from concourse.bass_utils import run_bass_kernel_spmd

    hidden = np.ascontiguousarray(np.asarray(hidden, dtype=np.float32))
    enc = np.ascontiguousarray(np.asarray(encoder_outputs, dtype=np.float32))
    W = np.ascontiguousarray(np.asarray(W, dtype=np.float32))

    nc = _get_nc()
    in_maps = []
    for i in range(NCORES):
        sl = slice(i * BPC, (i + 1) * BPC)
        in_maps.append(
            {
                "enc": np.ascontiguousarray(enc[sl]),
                "hT": np.ascontiguousarray(hidden[sl].T),
                "W": W,
            }
        )
    res = run_bass_kernel_spmd(nc, in_maps, core_ids=list(range(NCORES)), trace=trace)
    out = np.concatenate([r["out"] for r in res.results], axis=0)  # [B, S]
    return out[:, None, :].astype(np.float32), res


def kernel(hidden, encoder_outputs, W, b=None, **_ignored):
    out, _ = run(hidden, encoder_outputs, W)
    return out
